# revision 1
# baseline (speedup 1.0000x reference)
"""Block-circulant process via frequency-domain factorization on 8 cores.

out = x @ M factorizes through the (truncated, 48-bin) real FFT:
  stage A: per in-block j:  S[(p,f), b] = sum_t F[t,(p,f)] xT[jB+t, b]
  stage M: per freq pair e: mid[(q,i), b] = sum_{p,j} W_e[(p,j),(q,i)] S
  stage C: per out-block i: out[t, b] = sum_{q,f} G[(q,f), t] mid

All stages are single K<=128 matmuls (no PSUM accumulation). The two
partition-regroups between stages bounce through DRAM with affine
scatter APs. Sharding: pure data-parallel over batch (x dim 0), all
weight operands replicated. fp32r throughout.

PE per core: 88 matmuls (~20us). HBM per core: ~41 MiB.
"""

import numpy as np

B = 128
K_HALF = B // 2 + 1  # 65
KT = 48  # frequency truncation
KI = 32
KO = 32
BATCH = 4096
IN_F = 4096
OUT_F = 4096

N_CORES = 8
BQ = BATCH // N_CORES  # 512 batch rows per core
NP = KT // 2  # 24 frequency pairs
FE = NP  # e index range

_CACHE = {}
LAST_RESULTS = None
TRACE = False


def _build_nc():
    import concourse.bacc as bacc
    import concourse.mybir as mybir
    import concourse.tile as tile

    F32R = mybir.dt.float32r
    F32 = mybir.dt.float32

    nc = bacc.Bacc(None, target_bir_lowering=False)
    xT = nc.declare_dram_parameter("xT", [IN_F, BQ], F32R, isOutput=False)
    fmat = nc.declare_dram_parameter("fmat", [128, 96], F32R, isOutput=False)
    gmat = nc.declare_dram_parameter("gmat", [96, 128], F32R, isOutput=False)
    wmid = nc.declare_dram_parameter("wmid", [128, NP * 128], F32R,
                                     isOutput=False)
    oT = nc.declare_dram_parameter("oT", [OUT_F, BQ], F32, isOutput=True)

    # DRAM intermediates, laid out so stages M and C each load their whole
    # input with ONE contiguous DMA (48/64KB partition lines)
    # sS[fl*64 + p*32 + j, e*BQ + b]
    sS = nc.dram_tensor("sS", [128, NP * BQ], F32R)
    # cmid[q*48 + f, i*BQ + b]
    cmid = nc.dram_tensor("cmid", [96, KO * BQ], F32R)

    # views for the scattered writes
    sS_v = sS.rearrange("(fl p j) (e b) -> fl j p e b", fl=2, p=2, e=NP)
    cmid_v = cmid.rearrange("(q fe fl) (i b) -> fl fe q i b", fl=2, fe=FE,
                            i=KO)

    with tile.TileContext(nc) as tc:
        with (
            tc.tile_pool(name="cpool", bufs=1) as cpool,
            tc.tile_pool(name="xpool", bufs=8) as xpool,
            tc.tile_pool(name="spool", bufs=24) as spool,
            tc.tile_pool(name="bigpool", bufs=3) as bigpool,
            tc.tile_pool(name="opool", bufs=10) as opool,
            tc.tile_pool(name="psum", bufs=3, space="PSUM") as psum,
            tc.tile_pool(name="psum2", bufs=2, space="PSUM") as psum2,
        ):
            f_t = cpool.tile([128, 96], F32R, name="f_t")
            nc.sync.dma_start(f_t[:], fmat[:])
            g_t = cpool.tile([96, 128], F32R, name="g_t")
            nc.sync.dma_start(g_t[:], gmat[:])
            # all 24 middle weight blocks in one DMA
            w_all = cpool.tile([128, NP * 128], F32R, name="w_all")
            nc.sync.dma_start(w_all[:], wmid[:])

            lanes = [nc.scalar, nc.sync, nc.gpsimd]

            # ---- stage A: 32 matmuls + scattered DRAM writes ----
            for j in range(KI):
                x_t = xpool.tile([128, BQ], F32R, name="x_t")
                (nc.sync if j % 2 == 0 else nc.scalar).dma_start(
                    x_t[:], xT[j * 128:(j + 1) * 128, :])
                ps = psum.tile([96, BQ], mybir.dt.float32, name="ps_a",
                               tag="ps_a")
                nc.tensor.matmul(ps[:], f_t[:], x_t[:], start=True, stop=True)
                s_t = spool.tile([96, BQ], F32, name="s_t")
                nc.vector.tensor_copy(s_t[:], ps[:])
                for fl in range(2):
                    nc.gpsimd.dma_start(
                        sS_v[fl, j],
                        s_t[fl * 48:(fl + 1) * 48, :].bitcast(F32R),
                    )

            # ---- stage M: grouped reads (6 pairs/DMA) + 24 matmuls ----
            EG = 6
            for g in range(NP // EG):
                m_g = bigpool.tile([128, EG * BQ], F32R, name="m_g",
                                   tag="big")
                nc.sync.dma_start(m_g[:], sS[:, g * EG * BQ:(g + 1) * EG * BQ])
                for ee in range(EG):
                    e = g * EG + ee
                    ps = psum2.tile([128, BQ], mybir.dt.float32,
                                    name="ps_m", tag="ps_m")
                    nc.tensor.matmul(ps[:], w_all[:, e * 128:(e + 1) * 128],
                                     m_g[:, ee * BQ:(ee + 1) * BQ],
                                     start=True, stop=True)
                    m_out = opool.tile([128, BQ], F32, name="m_out", tag="mo")
                    nc.vector.tensor_copy(m_out[:], ps[:])
                    for fl in range(2):
                        (nc.scalar if fl == 0 else nc.gpsimd).dma_start(
                            cmid_v[fl, e],
                            m_out[fl * 64:(fl + 1) * 64, :].bitcast(F32R),
                        )

            # ---- stage C: grouped reads (8 i/DMA) + 32 matmuls ----
            IG = 8
            for g in range(KO // IG):
                c_g = bigpool.tile([96, IG * BQ], F32R, name="c_g", tag="big")
                nc.sync.dma_start(c_g[:],
                                  cmid[:, g * IG * BQ:(g + 1) * IG * BQ])
                for ii in range(IG):
                    i = g * IG + ii
                    ps = psum.tile([128, BQ], mybir.dt.float32, name="ps_c",
                                   tag="ps_c")
                    nc.tensor.matmul(ps[:], g_t[:],
                                     c_g[:, ii * BQ:(ii + 1) * BQ],
                                     start=True, stop=True)
                    o_t = opool.tile([128, BQ], F32, name="o_t")
                    nc.vector.tensor_copy(o_t[:], ps[:])
                    (nc.scalar if i % 2 == 0 else nc.gpsimd).dma_start(
                        oT[i * 128:(i + 1) * 128, :], o_t[:])
    nc.finalize()
    return nc


def _get_nc():
    if "nc" not in _CACHE:
        _CACHE["nc"] = _build_nc()
    return _CACHE["nc"]


def _host_weights(W_real, W_imag):
    """F [128,96], G [96,128], Wmid [24,128,128] (all float32)."""
    t = np.arange(B)[:, None].astype(np.float64)
    # F columns ordered (fl, p, e): f = 2e + fl; p=0 -> cos, p=1 -> -sin
    F = np.zeros((128, 96))
    for fl in range(2):
        for p in range(2):
            for e in range(FE):
                f = 2 * e + fl
                col = fl * 48 + p * 24 + e
                w = 2 * np.pi * f * t[:, 0] / B
                F[:, col] = np.cos(w) if p == 0 else -np.sin(w)
    # G rows ordered (q, f): q=0 -> scale*cos, q=1 -> -scale*sin
    G = np.zeros((96, 128))
    fs = np.arange(KT)
    scale = np.full(KT, 2.0 / B)
    scale[0] = 1.0 / B
    for q in range(2):
        for f in range(KT):
            w = 2 * np.pi * f * np.arange(B) / B
            G[q * 48 + f] = (scale[f] * np.cos(w) if q == 0
                             else -scale[f] * np.sin(w))
    # Wmid[e]: rows (fl, p, j), cols (fl, q, i); block-diag in fl
    Wr = W_real.astype(np.float64)
    Wi = W_imag.astype(np.float64)
    Wm = np.zeros((NP, 128, 128))
    for e in range(NP):
        for fl in range(2):
            f = 2 * e + fl
            r0, c0 = fl * 64, fl * 64
            # q=0: Re_out = Wr @ Re + Wi @ Im ; q=1: Im_out = Wr @ Im - Wi @ Re
            # rows (p=0: Re-in j), (p=1: Im-in j); cols (q, i)
            # lhsT[(p,j),(q,i)]: value multiplying S[p,j] into out[q,i]
            Wrf = Wr[:, :, f].T  # [j, i]
            Wif = Wi[:, :, f].T
            Wm[e, r0:r0 + 32, c0:c0 + 32] = Wrf          # p0 -> q0: Wr
            Wm[e, r0 + 32:r0 + 64, c0:c0 + 32] = Wif     # p1 -> q0: Wi
            Wm[e, r0:r0 + 32, c0 + 32:c0 + 64] = -Wif    # p0 -> q1: -Wi
            Wm[e, r0 + 32:r0 + 64, c0 + 32:c0 + 64] = Wrf  # p1 -> q1: Wr
    return (F.astype(np.float32), G.astype(np.float32), Wm.astype(np.float32))


def kernel(x, W_real, W_imag):
    global LAST_RESULTS
    from concourse.bass_utils import run_bass_kernel_spmd

    x = np.asarray(x, dtype=np.float32)
    F, G, Wm = _host_weights(np.asarray(W_real), np.asarray(W_imag))
    xt = np.ascontiguousarray(x.T)  # (IN_F, BATCH)

    in_maps = []
    for core in range(N_CORES):
        xT_shard = np.ascontiguousarray(xt[:, core * BQ:(core + 1) * BQ])
        wm_packed = np.ascontiguousarray(
            Wm.transpose(1, 0, 2).reshape(128, NP * 128))
        in_maps.append(
            {"xT": xT_shard, "fmat": F, "gmat": G, "wmid": wm_packed})

    nc = _get_nc()
    res = run_bass_kernel_spmd(nc, in_maps, list(range(N_CORES)), trace=TRACE)
    LAST_RESULTS = res

    out = np.empty((BATCH, OUT_F), np.float32)
    for core in range(N_CORES):
        out[core * BQ:(core + 1) * BQ, :] = res.results[core]["oT"].T
    return out



# revision 2
# speedup vs baseline: 1.3534x; 1.3534x over previous
"""Block-circulant process, frequency-domain factorization, 8 cores, bf16.

v4: single batch chunk; DRAM bounce with scatter-on-WRITE (posted writes
absorb the strided side) and fully-contiguous reads split in e/i-quarters
so stage M/C matmuls start as soon as the first quarter lands. F+G load
first (tiny) so stage A starts immediately; Wmid loads later, hidden
under stage A. PSUM->SBUF copies rotate over vector/scalar/gpsimd.
"""

import numpy as np
import ml_dtypes

B = 128
K_HALF = B // 2 + 1
KT = 48
KI = 32
KO = 32
BATCH = 4096
IN_F = 4096
OUT_F = 4096

N_CORES = 8
BQ = BATCH // N_CORES  # 512
NP = KT // 2  # 24

WCOL_W = NP * 128
WCOL_FG = 96 + 128

JG = 8

_CACHE = {}
LAST_RESULTS = None
TRACE = False


def _build_nc():
    import concourse.bacc as bacc
    import concourse.mybir as mybir
    import concourse.tile as tile

    BF16 = mybir.dt.bfloat16
    F32 = mybir.dt.float32

    nc = bacc.Bacc(None, target_bir_lowering=False)
    xT = nc.declare_dram_parameter("xT", [IN_F, BQ], BF16, isOutput=False)
    wfg = nc.declare_dram_parameter("wfg", [128, WCOL_FG], BF16,
                                    isOutput=False)
    wmid = nc.declare_dram_parameter("wmid", [128, WCOL_W], BF16,
                                     isOutput=False)
    oT = nc.declare_dram_parameter("oT", [OUT_F, BQ], BF16, isOutput=True)

    # bounce buffers laid out READ-optimally (reads fully contiguous);
    # writes scatter into them
    sD = nc.dram_tensor("sD", [128, NP * BQ], BF16)   # rows (fp j), cols (e b)
    cD = nc.dram_tensor("cD", [96, KO * BQ], BF16)    # rows (fq e), cols (i b)
    sD_v = sD.rearrange("(fp j) (e b) -> fp j e b", fp=4, e=NP)
    cD_v = cD.rearrange("(fq e) (i b) -> fq e i b", fq=4, i=KO)

    xT_v = xT.rearrange("(j t) b -> t j b", t=128)
    oT_v = oT.rearrange("(i t) b -> t i b", t=128)

    JH = KI // 2   # j-half for r1 writes
    EH = NP // 2   # e-half for r2 writes
    EQ = NP // 4   # e-quarter for r1 reads
    IQ = KO // 4   # i-quarter for r2 reads

    with tile.TileContext(nc) as tc:
        with (
            tc.tile_pool(name="cpool", bufs=1) as cpool,
            tc.tile_pool(name="xpool", bufs=1) as xpool,
            tc.tile_pool(name="spool", bufs=1) as spool,
            tc.tile_pool(name="mpool", bufs=1) as mpool,
            tc.tile_pool(name="midpool", bufs=1) as midpool,
            tc.tile_pool(name="crpool", bufs=1) as crpool,
            tc.tile_pool(name="opool", bufs=1) as opool,
            tc.tile_pool(name="psum", bufs=8, space="PSUM") as psum,
        ):
            x_t = xpool.tile([128, KI * BQ], BF16, name="x")
            S_t = spool.tile([96, KI * BQ], BF16, name="S")
            m_t = mpool.tile([128, NP * BQ], BF16, name="m")
            mid_t = midpool.tile([128, NP * BQ], BF16, name="mid")
            c_t = crpool.tile([96, KO * BQ], BF16, name="c")
            o_t = opool.tile([128, KO * BQ], BF16, name="o")

            # tiny F+G first so stage A can start right away
            fg = cpool.tile([128, WCOL_FG], BF16, name="fg")
            nc.sync.dma_start(fg[:], wfg[:])
            f_t = fg[:, 0:96]
            g_t = fg[0:96, 96:WCOL_FG]

            xv = x_t.rearrange("t (j b) -> t j b", j=KI)
            for g in range(KI // JG):
                nc.sync.dma_start(xv[:, g * JG:(g + 1) * JG, :],
                                  xT_v[:, g * JG:(g + 1) * JG, :])

            def copy(eng, dst, src):
                if eng is nc.scalar:
                    eng.copy(dst, src)
                else:
                    eng.tensor_copy(dst, src)

            engs = [nc.vector, nc.scalar]

            def r1_write(jh):
                js = slice(jh * JH, (jh + 1) * JH)
                cols = slice(jh * JH * BQ, (jh + 1) * JH * BQ)
                for fp in range(4):
                    nc.gpsimd.dma_start(
                        sD_v[fp, js].rearrange("j e b -> e j b"),
                        S_t[fp * 24:(fp + 1) * 24, cols])

            # ---- stage A + r1 scattered writes ----
            for j in range(KI):
                ps = psum.tile([128, BQ], F32, name="ps", tag="ps")
                nc.tensor.matmul(ps[0:96, :], f_t,
                                 x_t[:, j * BQ:(j + 1) * BQ],
                                 start=True, stop=True)
                copy(engs[j % 2], S_t[:, j * BQ:(j + 1) * BQ], ps[0:96, :])
                if j == JH - 1:
                    # Wmid load rides sync after x, before M needs it
                    w_t = cpool.tile([128, WCOL_W], BF16, name="w")
                    nc.sync.dma_start(w_t[:], wmid[:])
                if j == JH - 1 or j == KI - 1:
                    r1_write(j // JH)

            # r1 reads: contiguous e-quarters
            for q in range(4):
                cols = slice(q * EQ * BQ, (q + 1) * EQ * BQ)
                nc.sync.dma_start(m_t[:, cols], sD[:, cols])

            # ---- stage M + r2 scattered writes ----
            def r2_write(eh):
                es = slice(eh * EH, (eh + 1) * EH)
                cols = slice(eh * EH * BQ, (eh + 1) * EH * BQ)
                for fq in range(4):
                    nc.gpsimd.dma_start(
                        cD_v[fq, es].rearrange("e i b -> i e b"),
                        mid_t[fq * 32:(fq + 1) * 32, cols])

            for e in range(NP):
                ps = psum.tile([128, BQ], F32, name="ps", tag="ps")
                nc.tensor.matmul(ps[:], w_t[:, e * 128:(e + 1) * 128],
                                 m_t[:, e * BQ:(e + 1) * BQ],
                                 start=True, stop=True)
                copy(engs[e % 2], mid_t[:, e * BQ:(e + 1) * BQ], ps[:])
                if e == EH - 1 or e == NP - 1:
                    r2_write(e // EH)

            # r2 reads: contiguous i-quarters
            for q in range(4):
                cols = slice(q * IQ * BQ, (q + 1) * IQ * BQ)
                nc.sync.dma_start(c_t[:, cols], cD[:, cols])

            # ---- stage C + out stores ----
            ov = o_t.rearrange("t (i b) -> t i b", i=KO)
            for i in range(KO):
                ps = psum.tile([128, BQ], F32, name="ps", tag="ps")
                nc.tensor.matmul(ps[:], g_t, c_t[:, i * BQ:(i + 1) * BQ],
                                 start=True, stop=True)
                copy(engs[i % 2], o_t[:, i * BQ:(i + 1) * BQ], ps[:])
                if i % JG == JG - 1:
                    g = i // JG
                    nc.sync.dma_start(
                        oT_v[:, g * JG:(g + 1) * JG, :],
                        ov[:, g * JG:(g + 1) * JG, :])
    nc.finalize()
    return nc


def _get_nc():
    if "nc" not in _CACHE:
        _CACHE["nc"] = _build_nc()
    return _CACHE["nc"]


def _host_weights(W_real, W_imag):
    """F [128,96], G2 [96,128], Wmid [24,128,128] (float32, pre-bf16)."""
    t = np.arange(B).astype(np.float64)
    F = np.zeros((128, 96))
    for fl in range(2):
        for p in range(2):
            for e in range(NP):
                f = 2 * e + fl
                col = fl * 48 + p * 24 + e
                w = 2 * np.pi * f * t / B
                F[:, col] = np.cos(w) if p == 0 else -np.sin(w)
    G2 = np.zeros((96, 128))
    scale = np.full(KT, 2.0 / B)
    scale[0] = 1.0 / B
    for fl in range(2):
        for q in range(2):
            for e in range(NP):
                f = 2 * e + fl
                w = 2 * np.pi * f * np.arange(B) / B
                G2[fl * 48 + q * 24 + e] = (
                    scale[f] * np.cos(w) if q == 0 else -scale[f] * np.sin(w))
    Wr = W_real.astype(np.float64)
    Wi = W_imag.astype(np.float64)
    Wm = np.zeros((NP, 128, 128))
    for e in range(NP):
        for fl in range(2):
            f = 2 * e + fl
            r0 = fl * 64
            Wrf = Wr[:, :, f].T
            Wif = Wi[:, :, f].T
            Wm[e, r0:r0 + 32, r0:r0 + 32] = Wrf
            Wm[e, r0 + 32:r0 + 64, r0:r0 + 32] = Wif
            Wm[e, r0:r0 + 32, r0 + 32:r0 + 64] = -Wif
            Wm[e, r0 + 32:r0 + 64, r0 + 32:r0 + 64] = Wrf
    return (F.astype(np.float32), G2.astype(np.float32),
            Wm.astype(np.float32))


def _pack_inputs(x, W_real, W_imag):
    bf16 = ml_dtypes.bfloat16
    F, G2, Wm = _host_weights(np.asarray(W_real), np.asarray(W_imag))
    wfg = np.zeros((128, WCOL_FG), np.float32)
    wfg[:, :96] = F
    wfg[:96, 96:] = G2
    wmid = Wm.transpose(1, 0, 2).reshape(128, WCOL_W)
    xt = np.ascontiguousarray(np.asarray(x, np.float32).T.astype(bf16))
    return xt, wfg.astype(bf16), np.ascontiguousarray(wmid.astype(bf16))


def kernel(x, W_real, W_imag):
    global LAST_RESULTS
    from concourse.bass_utils import run_bass_kernel_spmd

    xt, wfg, wmid = _pack_inputs(x, W_real, W_imag)
    in_maps = []
    for core in range(N_CORES):
        in_maps.append({
            "xT": np.ascontiguousarray(xt[:, core * BQ:(core + 1) * BQ]),
            "wfg": wfg,
            "wmid": wmid,
        })

    nc = _get_nc()
    res = run_bass_kernel_spmd(nc, in_maps, list(range(N_CORES)), trace=TRACE)
    LAST_RESULTS = res

    out = np.empty((BATCH, OUT_F), np.float32)
    for core in range(N_CORES):
        out[core * BQ:(core + 1) * BQ, :] = \
            res.results[core]["oT"].T.astype(np.float32)
    return out
